# revision 21
# baseline (speedup 1.0000x reference)
"""DBRX MoE experts kernel for Trainium2 (8 NeuronCores).

Strategy (expert-parallel + fp8 DoubleRow):
  - Router (logits -> softmax -> top-2 -> renormalize) on host in numpy; it
    determines the token->expert dispatch.
  - Expert-parallel: core c owns expert c end-to-end (full FFN), processing
    the ~1030 tokens routed to it, padded to a uniform NB blocks of 384 so
    all 8 cores run the identical SPMD program. No collectives.
  - All matmuls run in fp8 e4m3 with MatmulPerfMode.DoubleRow (two k-rows
    per partition, 0.5 PE cycles per output row). Accuracy is recovered with
    a 3-term error-compensated product: for operands a ~ a_hi + a_lo and
    b ~ b_hi + b_lo (both split host- or device-side into two e4m3 levels at
    a shared power-of-two scale), a.b ~ a_hi.b_hi + a_lo.b_hi + a_hi.b_lo.
    Measured end-to-end rel err ~2e-3 (tolerance 2e-2).
  - Phase 1 (gate/up): per I-tile of 128 rows and 384-token block, psum
    accumulates 8 k-chunks x 3 terms of DoubleRow matmuls (x moving).
    ACT computes silu(gate); DVE computes h = silu(gate)*up scaled to fp8
    range, then h_hi = fp8(h), h_lo = fp8(h - h_hi).
  - Phase 2 (down): W2 moving in 512-wide D chunks, h stationary; psum
    [128 tokens, 512 D] accumulates 16 I-pairs x 3 terms; ACT evacuates with
    the per-token combine weight folded in; DMA straight to DRAM.
  - Host assembles out[t] = packed[e0][row0] + packed[e1][row1].
"""

import math

import numpy as np
import ml_dtypes

T = 4096
D = 2048
E = 8
I = 4096
NCORES = 8
BLKT = 384          # tokens per block (matmul moving free dim)
KCH = D // 256      # 8 k-chunks of 256 (DoubleRow pairs) for gate/up
ITILES = I // 128   # 32 I-tiles of 128 rows
IPAIR = I // 256    # 16 I-pair chunks for down proj
DCHK = D // 512     # 4 D-chunks of 512 for down proj

SX = 16.0           # x scale
SW = 1024.0         # W1 (gate/up) scale
SH = 8.0            # h scale
SW2 = 1024.0        # W2 scale

E4 = ml_dtypes.float8_e4m3

_CACHE: dict = {}


def _host_router(x, router_w):
    """Replicate reference routing in numpy (fp32)."""
    logits = (x.astype(np.float64) @ router_w.astype(np.float64).T).astype(np.float32)
    m = logits.max(axis=-1, keepdims=True)
    ex = np.exp((logits - m).astype(np.float32))
    probs = ex / ex.sum(axis=-1, keepdims=True)
    top1 = probs.argmax(axis=-1)
    p = probs.copy()
    p[np.arange(T), top1] = -1.0
    top2 = p.argmax(axis=-1)
    w1 = probs[np.arange(T), top1]
    w2 = probs[np.arange(T), top2]
    s = w1 + w2
    return top1.astype(np.int64), top2.astype(np.int64), (w1 / s).astype(np.float32), (w2 / s).astype(np.float32)


def _split_fp8(a, scale):
    """Two-level e4m3 split of a*scale: returns (hi, lo) fp8 arrays with
    a*scale ~ hi + lo."""
    s = (a * scale).astype(np.float32)
    hi = s.astype(E4)
    lo = (s - hi.astype(np.float32)).astype(E4)
    return hi, lo


def _build_bass(sizes: tuple):
    """8-core SPMD program; sizes = per-core token block sizes (equal stride
    BMAX = sizes[0]; last may be shorter)."""
    import concourse.bacc as bacc
    import concourse.mybir as mybir
    import concourse.tile as tile

    f32 = mybir.dt.float32
    f8 = mybir.dt.float8e4
    DR = mybir.MatmulPerfMode.DoubleRow
    Silu = mybir.ActivationFunctionType.Silu
    Copy = mybir.ActivationFunctionType.Copy
    mul_op = mybir.AluOpType.mult
    sub_op = mybir.AluOpType.subtract

    nb = len(sizes)
    bmax = sizes[0]
    ncols = nb * bmax                # h column space (block b at b*bmax)
    tsub = -(-(ncols) // 128)        # ceil
    ntok = tsub * 128                # h/out row space

    nc = bacc.Bacc("TRN2", target_bir_lowering=False)
    xh_d = nc.dram_tensor("xh", [128, nb, KCH, 2, bmax], f8, kind="ExternalInput")
    xl_d = nc.dram_tensor("xl", [128, nb, KCH, 2, bmax], f8, kind="ExternalInput")
    w1h_d = nc.dram_tensor("w1h", [128, ITILES, KCH, 2, 256], f8, kind="ExternalInput")
    w1l_d = nc.dram_tensor("w1l", [128, ITILES, KCH, 2, 256], f8, kind="ExternalInput")
    w2h_d = nc.dram_tensor("w2h", [128, DCHK, IPAIR, 2, 512], f8, kind="ExternalInput")
    w2l_d = nc.dram_tensor("w2l", [128, DCHK, IPAIR, 2, 512], f8, kind="ExternalInput")
    cw_d = nc.dram_tensor("cw", [128, tsub], f32, kind="ExternalInput")
    out_d = nc.dram_tensor("out", [ntok, D], f32, kind="ExternalOutput")

    IC = 1          # I-tiles per streamed W1 chunk
    NIC = ITILES // IC

    with tile.TileContext(nc) as tc:
        with (
            tc.tile_pool(name="xpool", bufs=1) as xpool,
            tc.tile_pool(name="hpool", bufs=1) as hpool,
            tc.tile_pool(name="wpool", bufs=2) as wpool,
            tc.tile_pool(name="sgpool", bufs=3) as sgpool,
            tc.tile_pool(name="hfpool", bufs=3) as hfpool,
            tc.tile_pool(name="evpool", bufs=3) as evpool,
            tc.tile_pool(name="const", bufs=1) as const_pool,
            tc.tile_pool(name="php", bufs=4, space="PSUM") as php,
            tc.tile_pool(name="pop", bufs=3, space="PSUM") as pop,
        ):
            # ---- initial DMAs: W1 chunk 0 (hi+lo), then x (hi+lo), cw ----
            w1_tiles = [None] * NIC  # (hi, lo) per chunk, allocated on demand
            w1h_t0 = wpool.tile([128, KCH, 2, 256], f8, tag="w1h")
            nc.sync.dma_start(w1h_t0[:], w1h_d[:, 0])
            xh_b = [
                xpool.tile([128, KCH, 2, bmax], f8, tag=f"xh{b}", name=f"xh{b}")
                for b in range(nb)
            ]
            xl_b = [
                xpool.tile([128, KCH, 2, bmax], f8, tag=f"xl{b}", name=f"xl{b}")
                for b in range(nb)
            ]
            nc.sync.dma_start(xh_b[0][:], xh_d[:, 0])
            w1l_t0 = wpool.tile([128, KCH, 2, 256], f8, tag="w1l")
            nc.sync.dma_start(w1l_t0[:], w1l_d[:, 0])
            nc.sync.dma_start(xl_b[0][:], xl_d[:, 0])
            for b in range(1, nb):
                nc.sync.dma_start(xh_b[b][:], xh_d[:, b])
                nc.sync.dma_start(xl_b[b][:], xl_d[:, b])
            w1_tiles[0] = (w1h_t0, w1l_t0)

            cw_sb = const_pool.tile([128, tsub], f32)
            nc.sync.dma_start(cw_sb[:], cw_d[:])

            hh = hpool.tile([128, ITILES, ntok], f8, tag="hh")
            hl = hpool.tile([128, ITILES, ntok], f8, tag="hl")

            # ---- phase 1: gate/up + h ----
            for ic in range(NIC):
                if w1_tiles[ic] is None:
                    w1h_t = wpool.tile([128, KCH, 2, 256], f8, tag="w1h")
                    nc.sync.dma_start(w1h_t[:], w1h_d[:, ic])
                    w1l_t = wpool.tile([128, KCH, 2, 256], f8, tag="w1l")
                    nc.sync.dma_start(w1l_t[:], w1l_d[:, ic])
                    w1_tiles[ic] = (w1h_t, w1l_t)
                w1h_t, w1l_t = w1_tiles[ic]
                for itl in range(IC):
                    it = ic
                    for b in range(nb):
                        sb = sizes[b]
                        hc0 = b * bmax
                        pg = php.tile([128, 512], f32, tag="ph", name=f"pg_{it}_{b}")
                        pu = php.tile([128, 512], f32, tag="ph", name=f"pu_{it}_{b}")
                        # terms: (xh,w1h), (xh,w1l), (xl,w1h)
                        terms = [(xh_b[b], w1h_t), (xh_b[b], w1l_t), (xl_b[b], w1h_t)]
                        n_mm = KCH * len(terms)
                        i_mm = 0
                        for (xt, wt) in terms:
                            for k in range(KCH):
                                st = i_mm == 0
                                sp = i_mm == n_mm - 1
                                nc.tensor.matmul(
                                    pg[:, :sb],
                                    wt[:, k, :, 0:128],
                                    xt[:, k, :, 0:sb],
                                    start=st, stop=sp, perf_mode=DR,
                                )
                                nc.tensor.matmul(
                                    pu[:, :sb],
                                    wt[:, k, :, 128:256],
                                    xt[:, k, :, 0:sb],
                                    start=st, stop=sp, perf_mode=DR,
                                )
                                i_mm += 1
                        sg = sgpool.tile([128, bmax], f32, tag="sg")
                        nc.scalar.activation(sg[:, :sb], pg[:, :sb], Silu, scale=1.0 / (SX * SW))
                        hf = hfpool.tile([128, bmax], f32, tag="hf")
                        # hf = (pu * SH/(SX*SW)) * sg  == SH * h
                        nc.vector.scalar_tensor_tensor(
                            hf[:, :sb], pu[:, :sb], SH / (SX * SW), sg[:, :sb], mul_op, mul_op
                        )
                        nc.scalar.activation(hh[:, it, hc0:hc0 + sb], hf[:, :sb], Copy)
                        # h_lo = hf - h_hi
                        nc.vector.scalar_tensor_tensor(
                            hl[:, it, hc0:hc0 + sb], hf[:, :sb], 1.0,
                            hh[:, it, hc0:hc0 + sb], mul_op, sub_op
                        )

            # ---- phase 2: down proj ----
            for dc in range(DCHK):
                w2h_t = wpool.tile([128, IPAIR, 2, 512], f8, tag="w2h")
                nc.sync.dma_start(w2h_t[:], w2h_d[:, dc])
                w2l_t = wpool.tile([128, IPAIR, 2, 512], f8, tag="w2l")
                nc.sync.dma_start(w2l_t[:], w2l_d[:, dc])
                for ts in range(tsub):
                    tt0, tt1 = ts * 128, (ts + 1) * 128
                    po = pop.tile([128, 512], f32, tag="po", name=f"po_{dc}_{ts}")
                    terms2 = [(hh, w2h_t), (hh, w2l_t), (hl, w2h_t)]
                    n_mm = IPAIR * len(terms2)
                    i_mm = 0
                    for (ht, wt) in terms2:
                        for q in range(IPAIR):
                            nc.tensor.matmul(
                                po[:],
                                ht[:, 2 * q:2 * q + 2, tt0:tt1],
                                wt[:, q],
                                start=(i_mm == 0), stop=(i_mm == n_mm - 1),
                                perf_mode=DR,
                            )
                            i_mm += 1
                    ev = evpool.tile([128, 512], f32, tag="ev")
                    nc.scalar.activation(ev[:], po[:], Copy, scale=cw_sb[:, ts:ts + 1])
                    nc.sync.dma_start(out_d[tt0:tt1, dc * 512:(dc + 1) * 512], ev[:])

    nc.compile()
    return nc


def _prepare(hidden_states, router_w, ws, w2s):
    x = np.asarray(hidden_states, dtype=np.float32).reshape(T, D)
    router_w = np.asarray(router_w, dtype=np.float32)
    ws = np.asarray(ws, dtype=np.float32)
    w2s = np.asarray(w2s, dtype=np.float32)

    top1, top2, w1, w2 = _host_router(x, router_w)

    toks: list[list[int]] = [[] for _ in range(E)]
    cws: list[list[float]] = [[] for _ in range(E)]
    for ti, wi in [(top1, w1), (top2, w2)]:
        for t in range(T):
            e = int(ti[t])
            toks[e].append(t)
            cws[e].append(float(wi[t]))

    max_n = max(len(tk) for tk in toks)
    nb = max(1, math.ceil(max_n / BLKT))
    bmax = math.ceil(max_n / nb)          # equal block sizes
    sizes = tuple([bmax] * (nb - 1) + [max_n - bmax * (nb - 1)])
    ncols = nb * bmax
    tsub = math.ceil(ncols / 128)
    ntok = tsub * 128

    # pos[k, t] = row of token t's k-th contribution in its expert's output
    pos = np.zeros((2, T), dtype=np.int64)
    expert_of = np.zeros((2, T), dtype=np.int64)
    seen = np.zeros(T, dtype=np.int64)
    for e in range(E):
        for j, t in enumerate(toks[e]):
            pos[seen[t], t] = j
            expert_of[seen[t], t] = e
            seen[t] += 1

    in_maps = []
    nxpad = nb * bmax
    for c in range(E):
        n = len(toks[c])
        perm = np.asarray(toks[c] + [0] * (nxpad - n), dtype=np.int64)
        xe = x[perm]
        if n < nxpad:
            xe[n:] = 0.0
        xhi, xlo = _split_fp8(xe, SX)  # [nxpad, D]
        # [128, nb, KCH, 2, bmax]: (p, b, k, j, t) = x[b*bmax + t, k*256 + j*128 + p]
        xh_a = np.ascontiguousarray(
            xhi.reshape(nb, bmax, KCH, 2, 128).transpose(4, 0, 2, 3, 1))
        xl_a = np.ascontiguousarray(
            xlo.reshape(nb, bmax, KCH, 2, 128).transpose(4, 0, 2, 3, 1))

        gate = ws[c, :I, :]   # [I, D]
        up = ws[c, I:, :]
        g_hi, g_lo = _split_fp8(gate, SW)
        u_hi, u_lo = _split_fp8(up, SW)

        def w1_layout(g, u):
            # [128, ITILES, KCH, 2, 256]: (p, it, k, j, m) =
            #   {gate,up}[it*128 + (m%128), k*256 + j*128 + p]
            g4 = g.reshape(ITILES, 128, KCH, 2, 128).transpose(4, 0, 2, 3, 1)
            u4 = u.reshape(ITILES, 128, KCH, 2, 128).transpose(4, 0, 2, 3, 1)
            return np.ascontiguousarray(np.concatenate([g4, u4], axis=4))

        w1h_a = w1_layout(g_hi, u_hi)
        w1l_a = w1_layout(g_lo, u_lo)

        w2T = w2s[c].T  # [I, D]
        w2_hi, w2_lo = _split_fp8(w2T, SW2)

        def w2_layout(w):
            # [128, DCHK, IPAIR, 2, 512]: (p, dc, q, j, d) =
            #   w2T[q*256 + j*128 + p, dc*512 + d]
            w4 = w.reshape(IPAIR, 2, 128, DCHK, 512).transpose(2, 3, 0, 1, 4)
            return np.ascontiguousarray(w4)

        w2h_a = w2_layout(w2_hi)
        w2l_a = w2_layout(w2_lo)

        cw = np.zeros(ntok, dtype=np.float32)
        cw[:n] = np.asarray(cws[c], dtype=np.float32)
        cw_a = np.ascontiguousarray(
            (cw / (SH * SW2)).reshape(tsub, 128).T)  # [128, tsub]

        in_maps.append({
            "xh": xh_a, "xl": xl_a,
            "w1h": w1h_a, "w1l": w1l_a,
            "w2h": w2h_a, "w2l": w2l_a,
            "cw": cw_a,
        })

    return sizes, pos, expert_of, in_maps


def kernel(hidden_states, router_w, ws, w2s):
    from concourse import bass_utils

    hs = np.asarray(hidden_states)
    B, S, _ = hs.shape
    sizes, pos, expert_of, in_maps = _prepare(hidden_states, router_w, ws, w2s)

    if sizes not in _CACHE:
        _CACHE[sizes] = _build_bass(sizes)
    nc = _CACHE[sizes]

    res = bass_utils.run_bass_kernel_spmd(nc, in_maps, core_ids=list(range(NCORES)))
    outs = [res.results[c]["out"] for c in range(NCORES)]  # [ntok, D] each

    out = np.zeros((T, D), dtype=np.float32)
    for k in range(2):
        e_arr = expert_of[k]
        p_arr = pos[k]
        for e in range(E):
            mask = e_arr == e
            out[mask] += outs[e][p_arr[mask]]
    return out.reshape(B, S, D).astype(np.float32)
